# revision 38
# baseline (speedup 1.0000x reference)
"""Multi-head attention (B=2, S=2048, D=1024, H=16, hd=64, RoPE, causal)
on 8 Trainium2 NeuronCores.

Sharding: each core owns 2 heads x both batches (tensor-parallel over heads).
Per core, everything is computed in transposed [feature, seq] layout with
bf16 matmuls:
  - Q/K/V projections from pre-transposed x (QT/KT/VT = W.T-slice.T @ x.T)
  - RoPE on QT/KT via a PE permutation matmul + 3 DVE ops
  - scores computed TRANSPOSED: ST[k,q] = KT_h.T @ QT_h, so softmax needs no
    max-subtraction (scores bounded) and no P-transpose; causal handled by
    loop bounds + one static triangle tile on diagonal blocks
  - exp on ACT with fused 1/sqrt(hd) scale; denominator via a ones-column
    appended to V (65th lane of the attn@V accumulation); reciprocal on DVE
  - re-shard heads->sequence via TWO per-batch 8-core AllToAlls so the first
    one overlaps batch-1 compute; out-projection run once per collective
    (ytq_a / ytq_b), host picks the valid quarter per core.

Perf notes (vs the original version):
  - DMA instructions are expensive (~1.2us of shared descriptor-gen each);
    all inputs are host-packed so startup is ~11 DMAs instead of ~107.
  - ones-lanes built with DVE memset instead of stride-0 broadcast DMAs.
  - attention emits homogeneous matmul runs (all scores, then all attnV)
    to keep the PE streaming; ACT does nothing but exp during attention.
"""
import os

import ml_dtypes
import numpy as np

import concourse.bass as bass
import concourse.mybir as mybir
import concourse.tile as tile
from concourse.bass_utils import run_bass_kernel_spmd
from concourse.vector_clock import ScopedClock

B, S, D, H, HD = 2, 2048, 1024, 16, 64
NCORES = 8
HPC = 2                    # heads per core
F = HPC * HD               # 128 features per core
CHUNK = 512
NCH = S // CHUNK           # 4 q-chunks
NKT = D // 128             # 8 contraction tiles for projections
NST = S // 128             # 16 key tiles
MASKVAL = -240.0           # -30 after the 1/8 softmax scale; exp(-30) ~ 1e-13
F32 = mybir.dt.float32
F32R = mybir.dt.float32r
BF16 = mybir.dt.bfloat16
NPBF16 = ml_dtypes.bfloat16

# consts tile column layouts
C_PERM = 0          # bf16 tensor: perm | ident | sel | chat
C_IDENT = 128
C_SEL = 256         # [8, 8*64]: block g = ones in row g (den broadcast)
C_CHAT = 256 + NCORES * 64
C_TOT = 256 + NCORES * 64 + S
F_MASK = 0          # f32 tensor: mask | shat
F_SHAT = 128
F_TOT = 128 + S


# ---------------------------------------------------------------------------
# Workarounds for the walrus build in this container: it encodes at most ONE
# sync-wait per instruction ("Too many sync wait commands"). Split multi-wait
# instructions into single-wait NoOps. Semantics-preserving.
# ---------------------------------------------------------------------------
_patched = False


def _install_patches():
    global _patched
    if _patched:
        return
    _patched = True

    _orig_lower = tile.TileContext._lower_ordered_insts

    def _lower_with_wait_split(self, ordered):
        nc = self.nc
        for _bb, insts in ordered.items():
            if not any(
                i.sync_info is not None and len(i.sync_info.on_wait) > 1
                for i in insts
            ):
                continue
            new = []
            for inst in insts:
                si = inst.sync_info
                if si is not None and len(si.on_wait) > 1:
                    waits = list(si.on_wait)
                    for w in waits[:-1]:
                        n = mybir.InstNoOp(
                            name=f"I-waitsplit-{nc.next_id()}", ins=[], outs=[]
                        )
                        n.engine = inst.engine
                        n.bass_nofuse = True
                        n.sync_info = mybir.SyncInfo(on_wait=[w], on_update=[])
                        nc.register_instruction(n)
                        new.append(n)
                    inst.sync_info = mybir.SyncInfo(
                        on_wait=[waits[-1]], on_update=list(si.on_update)
                    )
                new.append(inst)
            insts[:] = new
        return _orig_lower(self, ordered)

    tile.TileContext._lower_ordered_insts = _lower_with_wait_split

    def _drain_and_barrier(self, tick_clock, wait_clock):
        nc = self.nc
        probe = nc.sync.nop(nofuse=True)
        wait_clock.add_sem_waits(
            probe.ins, ScopedClock({None: tick_clock.global_clock})
        )
        waits = list(probe.ins.sync_info.on_wait)
        probe.ins.sync_info = mybir.SyncInfo(on_wait=waits[:1], on_update=[])
        for w in waits[1:]:
            n2 = nc.sync.nop(nofuse=True)
            n2.ins.sync_info = mybir.SyncInfo(on_wait=[w], on_update=[])
        nc.sync.drain()
        nc.all_engine_barrier()
        assert self.sems is not None
        popped = nc._tile_sem_poison_stack.pop()
        assert popped is self._sem_poison
        nc.clear_and_free_semaphores(list(self.sems.allocated().values()))
        nc.all_engine_barrier()

    tile.TileContext._drain_and_barrier = _drain_and_barrier


def _install_ntff_hook():
    """Provide the missing ``antenv.axon_hooks`` module so trace=True works."""
    import sys
    import types

    if "antenv.axon_hooks" in sys.modules:
        return
    try:
        import antenv
        from trn_agent_boot.trn_boot import _ntff_profile_via_ctypes
    except ImportError:
        return
    mod = types.ModuleType("antenv.axon_hooks")
    mod._hook = _ntff_profile_via_ctypes("/opt/axon/libaxon_pjrt.so")
    mod.set_axon_ntff_profile_hook = lambda h: setattr(mod, "_hook", h)
    mod.get_axon_ntff_profile_hook = lambda: mod._hook
    sys.modules["antenv.axon_hooks"] = mod
    antenv.axon_hooks = mod


# ---------------------------------------------------------------------------
# Program builder (same program on all 8 cores; per-core data differs)
# ---------------------------------------------------------------------------
def build_program():
    _install_patches()
    nc = bass.Bass(num_devices=NCORES)

    xt = [nc.dram_tensor(f"xt{b}", [D, S], BF16, kind="ExternalInput")
          for b in range(B)]
    wqkv = nc.dram_tensor("wqkv", [D, 3 * F], BF16, kind="ExternalInput")
    wot = nc.dram_tensor("wot", [D, D], BF16, kind="ExternalInput")
    consts = nc.dram_tensor("consts", [128, C_TOT], BF16, kind="ExternalInput")
    constsf = nc.dram_tensor("constsf", [128, F_TOT], F32, kind="ExternalInput")
    biasp = nc.dram_tensor("biasp", [128, 3 + NKT], F32, kind="ExternalInput")
    ytq_a = nc.dram_tensor("ytq_a", [D, CHUNK], BF16, kind="ExternalOutput")
    ytq_b = nc.dram_tensor("ytq_b", [D, CHUNK], BF16, kind="ExternalOutput")

    # a2a row layout: 128 blocks of CHUNK attn values (one per feature row,
    # UNNORMALIZED) followed by 2 blocks of CHUNK softmax denominators
    ROWX = F + HPC
    a2a_in = [nc.dram_tensor(f"a2a_in{b}", [NCORES, ROWX * CHUNK], BF16)
              for b in range(B)]
    a2a_out = [nc.dram_tensor(f"a2a_out{b}", [NCORES, ROWX * CHUNK], BF16)
               for b in range(B)]
    a2a_in3 = [t.rearrange("g (x n) -> g x n", n=CHUNK) for t in a2a_in]

    debug = bool(int(os.environ.get("MHA_DEBUG", "0")))
    if debug:
        dbg_qt = nc.dram_tensor("dbg_qt", [F, S], BF16, kind="ExternalOutput")
        dbg_kt = nc.dram_tensor("dbg_kt", [F, S], BF16, kind="ExternalOutput")
        dbg_vt = nc.dram_tensor("dbg_vt", [F, S], BF16, kind="ExternalOutput")
        dbg_a2a = nc.dram_tensor("dbg_a2a", [NCORES, (F + HPC) * CHUNK],
                                 BF16, kind="ExternalOutput")
        dbg_a2ao = [nc.dram_tensor(f"dbg_a2ao{b}", [NCORES, (F + HPC) * CHUNK],
                                   BF16, kind="ExternalOutput")
                    for b in range(B)]
        dbg_den = nc.dram_tensor("dbg_den", [NCORES, HPC * CHUNK], BF16,
                                 kind="ExternalOutput")
        dbg_denr = nc.dram_tensor("dbg_denr", [NCORES, HPC * CHUNK], BF16,
                                  kind="ExternalOutput")
        dbg_at2n = nc.dram_tensor("dbg_at2n", [128, NCORES * CHUNK], BF16,
                                  kind="ExternalOutput")

    with tile.TileContext(nc) as tc:
        with (
            tc.tile_pool(name="const", bufs=1) as constp,
            tc.tile_pool(name="wpool", bufs=1) as wpool,
            tc.tile_pool(name="xtp", bufs=1) as xtp,
            tc.tile_pool(name="raw", bufs=2) as rawp,
            tc.tile_pool(name="ropetmp", bufs=2) as ropetmp,
            tc.tile_pool(name="qkv", bufs=2) as qkv,
            tc.tile_pool(name="vagg", bufs=2) as vaggp,
            tc.tile_pool(name="expp", bufs=10) as expp,
            tc.tile_pool(name="normp", bufs=2) as normp,
            tc.tile_pool(name="stage", bufs=2) as stage,
            tc.tile_pool(name="at2", bufs=1) as at2p,
            tc.tile_pool(name="ys", bufs=1) as ysp,
            tc.tile_pool(name="ps", bufs=2, space="PSUM") as ps,
            tc.tile_pool(name="pav", bufs=2, space="PSUM") as pav,
            tc.tile_pool(name="pbp", bufs=1, space="PSUM") as pbp,
        ):
            # ---- input loads: few, large DMAs (desc-gen is expensive);
            # ---- issue order = first-needed first ----
            wqkv_t = constp.tile([128, NKT * 3 * F], BF16)
            nc.sync.dma_start(
                out=wqkv_t.rearrange("p (t f) -> p t f", t=NKT),
                in_=wqkv.rearrange("(t p) f -> p t f", t=NKT),
            )
            xt_t = {}

            def load_xt(b, c):
                cs = slice(CHUNK * c, CHUNK * (c + 1))
                t = xtp.tile([128, NKT * CHUNK], BF16, tag=f"xt{b}{c}",
                             name=f"xt{b}{c}")
                nc.sync.dma_start(
                    out=t.rearrange("p (t c) -> p t c", t=NKT),
                    in_=xt[b].rearrange("(t p) s -> p t s", t=NKT)[:, :, cs],
                )
                xt_t[(b, c)] = t

            load_xt(0, 0)
            consts_t = constp.tile([128, C_TOT], BF16)
            nc.sync.dma_start(out=consts_t, in_=consts[:])
            constsf_t = constp.tile([128, F_TOT], F32)
            nc.sync.dma_start(out=constsf_t, in_=constsf[:])
            biasp_t = constp.tile([128, 3 + NKT], F32)
            nc.sync.dma_start(out=biasp_t, in_=biasp[:])
            for c in range(1, NCH):
                load_xt(0, c)
            for c in range(NCH):
                load_xt(1, c)
            perm = consts_t[:, C_PERM:C_PERM + 128]
            ident = consts_t[:, C_IDENT:C_IDENT + 128]
            selg = consts_t[0:8, C_SEL:C_SEL + NCORES * 64]
            chat = consts_t[:, C_CHAT:C_CHAT + S]
            mask = constsf_t[:, F_MASK:F_MASK + 128]
            shat = constsf_t[:, F_SHAT:F_SHAT + S]

            wo_t = constp.tile([128, NKT * D], BF16)
            nc.sync.dma_start(
                out=wo_t.rearrange("p (g e) -> p g e", g=NKT),
                in_=wot.rearrange("(g p) e -> p g e", g=NKT),
            )

            # ones row for the denominator broadcast matmul (no DMA)
            ones_r = constp.tile([1, 64], BF16)
            nc.vector.memset(ones_r, 1.0)

            # per-head KT, zero-padded to the full 128 partitions so the
            # scores matmul drives the full PE array (half-height
            # stationaries stream at roughly half rate on TRN2)
            ktp = [qkv.tile([F, S], BF16, tag=f"ktp{h}", bufs=1,
                            name=f"ktp{h}")
                   for h in range(HPC)]
            nc.vector.memset(ktp[0][64:128, :], 0.0)
            nc.vector.memset(ktp[1][0:64, :], 0.0)

            for b in range(B):
                QT = qkv.tile([F, S], BF16, tag="QT")
                VT = qkv.tile([F, S], BF16, tag="VT")
                # vagg layout [128 keys, st, h, 128]: cols 0:64 = V rows,
                # col 64 = ones (denominator), cols 65:128 = zero padding so
                # the attnV stationary is a full 128x128 tile
                vagg = vaggp.tile([128, NST, HPC, 128], BF16, tag="vagg")
                nc.vector.memset(vagg[:, :, :, 65:128], 0.0)
                nc.vector.memset(vagg[:, :, :, 64], 1.0)

                # ---- projections + rope, chunk by chunk; PSUM is
                # ---- packed as 2-bank "wide" tiles: [proj | rope-perm] ----
                for c in range(NCH):
                    cs = slice(CHUNK * c, CHUNK * (c + 1))
                    xt_c = xt_t[(b, c)]
                    for pi, dst in ((0, QT), (1, None), (2, VT)):
                        wt = ps.tile([F, 2 * CHUNK], F32, tag="wide",
                                     name=f"w_proj{pi}")
                        pm = wt[:, 0:CHUNK]
                        for k in range(NKT):
                            nc.tensor.matmul(
                                pm,
                                wqkv_t[:, 3 * F * k + F * pi:
                                       3 * F * k + F * (pi + 1)],
                                xt_c[:, CHUNK * k:CHUNK * (k + 1)],
                                start=(k == 0), stop=(k == NKT - 1),
                            )
                        if pi == 2:
                            # bias folded here; no rope for V
                            nc.scalar.activation(
                                VT[:, cs], pm,
                                mybir.ActivationFunctionType.Identity,
                                bias=biasp_t[:, pi:pi + 1],
                            )
                            # V transposes into the spare half (bf16 view)
                            ptv = wt.bitcast(BF16)
                            for i, st in enumerate(range(4 * c, 4 * c + 4)):
                                pt = ptv[:, 2 * CHUNK + 128 * i:
                                         2 * CHUNK + 128 * (i + 1)]
                                nc.tensor.transpose(
                                    pt, VT[:, 128 * st:128 * (st + 1)], ident,
                                )
                                nc.scalar.activation(
                                    vagg[:, st, :, 0:64],
                                    pt.rearrange("p (h u) -> p h u", h=HPC),
                                    mybir.ActivationFunctionType.Copy,
                                )
                        else:
                            rawt = rawp.tile([F, CHUNK], BF16, tag="rawqk")
                            nc.scalar.activation(
                                rawt, pm,
                                mybir.ActivationFunctionType.Identity,
                                bias=biasp_t[:, pi:pi + 1],
                            )
                            # rope: dst = raw*Chat + swap32(raw)*Shat,
                            # swap32 done as a PE permutation matmul
                            psw = wt[:, CHUNK:2 * CHUNK]
                            nc.tensor.matmul(psw, perm, rawt,
                                             start=True, stop=True,
                                             skip_group_check=True)
                            t1 = ropetmp.tile([F, CHUNK], BF16, tag="t1")
                            nc.vector.tensor_mul(t1, rawt, chat[:, cs])
                            t2 = ropetmp.tile([F, CHUNK], BF16, tag="t2")
                            nc.vector.tensor_mul(t2, psw, shat[:, cs])
                            if dst is None:
                                # K lands in the per-head zero-padded tiles
                                for h in range(HPC):
                                    hs = slice(64 * h, 64 * (h + 1))
                                    nc.vector.tensor_add(
                                        ktp[h][hs, cs], t1[hs, :], t2[hs, :]
                                    )
                            else:
                                nc.vector.tensor_add(dst[:, cs], t1, t2)

                if debug and b == 0:
                    nc.sync.dma_start(out=dbg_qt[:], in_=QT[:])
                    nc.sync.dma_start(out=dbg_kt[:], in_=ktp[0][:])
                    nc.sync.dma_start(out=dbg_vt[:], in_=VT[:])

                # ---- attention: homogeneous runs of scores then attnV ----
                for c in range(NCH):
                    nkt = 4 * c + 4
                    stg = stage.tile([F, CHUNK], BF16, tag="stg")
                    stg_gate = stg
                    den_stg = [stage.tile([1, CHUNK], BF16, tag=f"den_stg{hh}",
                                          name=f"den_stg{hh}")
                               for hh in range(HPC)]
                    for h in range(HPC):
                        hs = slice(64 * h, 64 * (h + 1))
                        # scores run: kt-tile PAIRS share one 2-bank psum
                        # tile and ONE exp instruction (ACT per-instruction
                        # overhead is significant)
                        exs = []
                        for j in range(nkt // 2):
                            pm = ps.tile([128, 2 * CHUNK], F32, tag="wide",
                                         name="pm_scores")
                            ex = expp.tile([128, 2 * CHUNK], BF16, tag="exp")
                            info = []
                            for u in range(2):
                                kt = 2 * j + u
                                qlo = max(CHUNK * c, 128 * kt)
                                w = CHUNK * (c + 1) - qlo
                                off = CHUNK * u
                                nc.tensor.matmul(
                                    pm[:, off:off + w],
                                    ktp[h][:, 128 * kt:128 * (kt + 1)],
                                    QT[:, qlo:qlo + w],
                                    start=True, stop=True,
                                    skip_group_check=True,
                                )
                                if 128 * kt >= CHUNK * c:
                                    nc.vector.tensor_add(
                                        pm[:, off:off + 128],
                                        pm[:, off:off + 128], mask
                                    )
                                info.append((kt, qlo, w, off))
                            wl = info[1][2]
                            nc.scalar.activation(
                                ex[:, 0:CHUNK + wl], pm[:, 0:CHUNK + wl],
                                mybir.ActivationFunctionType.Exp,
                                scale=0.125,
                            )
                            exs.append((ex, info))
                        # attnV run: rows 0:64 = hd, row 64 = denominator,
                        # rows 65:128 = zero padding (full-width stationary)
                        av = pav.tile([128, CHUNK], F32, tag=f"av{h}", bufs=1)
                        for ex, info in exs:
                            for kt, qlo, w, off in info:
                                nc.tensor.matmul(
                                    av[:, qlo - CHUNK * c:CHUNK],
                                    vagg[:, kt, h, :],
                                    ex[:, off:off + w],
                                    start=(kt == 0), stop=(kt == nkt - 1),
                                )
                        # stage raw attn rows + denominator; normalization
                        # happens on the receiving core after the all-to-all
                        nc.vector.tensor_copy(stg[hs, :], av[0:64, :])
                        nc.vector.tensor_copy(den_stg[h], av[64:65, :])
                    nc.sync.dma_start(out=a2a_in3[b][4 * b + c, 0:F],
                                      in_=stg)
                    for hh in range(HPC):
                        nc.sync.dma_start(
                            out=a2a_in3[b][4 * b + c, F + hh:F + hh + 1],
                            in_=den_stg[hh],
                        )

                # ---- per-batch all-to-all: heads -> sequence quarters ----
                nc.gpsimd.collective_compute(
                    "AllToAll",
                    mybir.AluOpType.bypass,
                    replica_groups=[list(range(NCORES))],
                    ins=[a2a_in[b][:]],
                    outs=[a2a_out[b][:]],
                )

            if debug:
                nc.sync.dma_start(out=dbg_a2a[:], in_=a2a_in[0][:])
                for b in range(B):
                    nc.sync.dma_start(out=dbg_a2ao[b][:], in_=a2a_out[b][:])

            # ---- out projections (one per collective; host picks) ----
            # priority-demote so the Tile scheduler cannot hoist this
            # cc-dependent chain ahead of batch-1 attention
            for b, ytq in ((0, ytq_a), (1, ytq_b)):
                tc.cur_priority += 10 ** 6
                at2 = at2p.tile([128, NCORES * CHUNK], BF16, tag="at2",
                                name="at2")
                den_t = at2p.tile([NCORES, HPC * CHUNK], BF16,
                                  tag="den_t", name="den_t")
                # dummy writes gate the loads on the LAST staging tile so the
                # scheduler sequences this cc-dependent chain after attention
                # on every engine (its sim underestimates collective time)
                nc.vector.tensor_copy(at2[:, -1:], stg_gate[:, 0:1])
                nc.vector.tensor_copy(den_t[:, -1:], stg_gate[0:NCORES, 0:1])
                a2o = a2a_out[b].rearrange("g (x n) -> x g n", n=CHUNK)
                nc.sync.dma_start(
                    out=at2.rearrange("p (g n) -> p g n", g=NCORES),
                    in_=a2o[0:F],
                )
                nc.sync.dma_start(
                    out=den_t,
                    in_=a2a_out[b][:, F * CHUNK:(F + HPC) * CHUNK],
                )
                den32 = at2p.tile([NCORES, HPC * CHUNK], F32,
                                  tag="den32", name="den32")
                nc.vector.tensor_copy(den32, den_t)
                denr_t = at2p.tile([NCORES, HPC * CHUNK], BF16,
                                   tag="denr_t", name="denr_t")
                with nc.allow_low_precision(reason="softmax denom"):
                    nc.vector.reciprocal(denr_t, den32)
                # normalize each (source core, head) block of at2 in place;
                # the selector stationary (ones in row g) broadcasts denr row
                # g's head-h half to 64 output partitions
                for g in range(NCORES):
                    sl = slice(CHUNK * g, CHUNK * (g + 1))
                    pb = pbp.tile([128, CHUNK], F32, tag="pb", bufs=2)
                    for h in range(HPC):
                        nc.tensor.matmul(
                            pb[64 * h:64 * (h + 1), :],
                            selg[:, 64 * g:64 * (g + 1)],
                            denr_t[:, CHUNK * h:CHUNK * (h + 1)],
                            start=True, stop=True)
                    pbs = normp.tile([128, CHUNK], BF16, tag="pbs")
                    nc.vector.tensor_copy(pbs, pb)
                    nc.vector.tensor_mul(at2[:, sl], at2[:, sl], pbs)
                if debug and b == 0:
                    nc.sync.dma_start(out=dbg_den[:], in_=den_t)
                    nc.sync.dma_start(out=dbg_denr[:], in_=denr_t)
                    nc.sync.dma_start(out=dbg_at2n[:], in_=at2)
                ysb = ysp.tile([128, NKT * CHUNK], BF16, tag="ysb")
                for et in range(NKT):
                    if et % 2 == 0:
                        wy = ps.tile([128, 2 * CHUNK], F32, tag="wide",
                                     name="pm_yproj")
                    pm = wy[:, CHUNK * (et % 2):CHUNK * (et % 2 + 1)]
                    for g in range(NKT):
                        nc.tensor.matmul(
                            pm,
                            wo_t[:, D * g + 128 * et:D * g + 128 * (et + 1)],
                            at2[:, CHUNK * g:CHUNK * (g + 1)],
                            start=(g == 0), stop=(g == NKT - 1),
                        )
                    nc.vector.tensor_scalar_add(
                        ysb[:, CHUNK * et:CHUNK * (et + 1)], pm,
                        biasp_t[:, 3 + et:4 + et],
                    )
                nc.sync.dma_start(
                    out=ytq.rearrange("(e p) n -> p e n", e=NKT),
                    in_=ysb.rearrange("p (e n) -> p e n", e=NKT),
                )

    nc.finalize()
    return nc


_NC_CACHE = None


def _get_program():
    global _NC_CACHE
    if _NC_CACHE is None:
        _NC_CACHE = build_program()
    return _NC_CACHE


def _prep_in_maps(x, cos, sin, Wq, bq, Wk, bk, Wv, bv, Wo, bo):
    cosT = np.ascontiguousarray(cos.T).astype(np.float32)    # (32, S)
    sinT = np.ascontiguousarray(sin.T).astype(np.float32)
    chat = np.concatenate([cosT, cosT, cosT, cosT], 0)       # (128, S)
    shat = np.concatenate([-sinT, sinT, -sinT, sinT], 0)
    xT = [np.ascontiguousarray(x[b].T).astype(NPBF16) for b in range(B)]
    mask128 = np.where(np.arange(128)[:, None] > np.arange(128)[None, :],
                       np.float32(MASKVAL), np.float32(0.0))
    sw = np.arange(128)
    sw = np.where((sw // 32) % 2 == 0, sw + 32, sw - 32)
    perm128 = np.zeros((128, 128), np.float32)
    perm128[sw, np.arange(128)] = 1.0
    sel = np.zeros((128, NCORES * 64), np.float32)
    for g in range(NCORES):
        sel[g, 64 * g:64 * (g + 1)] = 1.0
    consts = np.concatenate(
        [perm128, np.eye(128, dtype=np.float32), sel, chat], axis=1
    ).astype(NPBF16)
    constsf = np.ascontiguousarray(
        np.concatenate([mask128, shat], axis=1).astype(np.float32)
    )

    wqT, wkT, wvT = (np.ascontiguousarray(W.T) for W in (Wq, Wk, Wv))
    woT = np.ascontiguousarray(Wo.T).astype(NPBF16)
    biasp = np.concatenate(
        [bq.reshape(NCORES, F, 1), bk.reshape(NCORES, F, 1),
         bv.reshape(NCORES, F, 1),
         np.broadcast_to(bo.reshape(1, NKT, 128).transpose(0, 2, 1),
                         (NCORES, 128, NKT))],
        axis=2,
    ).astype(np.float32)   # (NCORES, 128, 3+NKT), per-core slice

    in_maps = []
    for core in range(NCORES):
        sl = slice(F * core, F * (core + 1))
        wqkv = np.concatenate(
            [wqT[:, sl].reshape(D, 1, F), wkT[:, sl].reshape(D, 1, F),
             wvT[:, sl].reshape(D, 1, F)], axis=1
        ).reshape(D, 3 * F).astype(NPBF16)
        in_maps.append({
            "xt0": xT[0], "xt1": xT[1],
            "wqkv": np.ascontiguousarray(wqkv),
            "wot": woT,
            "consts": consts,
            "constsf": constsf,
            "biasp": np.ascontiguousarray(biasp[core]),
        })
    return in_maps


def kernel(x, cos, sin, mask, Wq, bq, Wk, bk, Wv, bv, Wo, bo, **_unused):
    """Full inputs in, full output out. `mask` (the causal mask) is
    regenerated on-device, so the input tensor itself is unused."""
    x, cos, sin = (np.asarray(a, np.float32) for a in (x, cos, sin))
    Wq, bq, Wk, bk = (np.asarray(a, np.float32) for a in (Wq, bq, Wk, bk))
    Wv, bv, Wo, bo = (np.asarray(a, np.float32) for a in (Wv, bv, Wo, bo))

    nc = _get_program()
    in_maps = _prep_in_maps(x, cos, sin, Wq, bq, Wk, bk, Wv, bv, Wo, bo)

    trace = bool(int(os.environ.get("MHA_TRACE", "0")))
    kw = {}
    if trace:
        _install_ntff_hook()
        kw = dict(trace=True, trace_cores=list(range(NCORES)))
    res = run_bass_kernel_spmd(nc, in_maps, core_ids=list(range(NCORES)), **kw)
    kernel.last_results = res

    y = np.empty((B, S, D), np.float32)
    for r in range(NCORES):
        b, c = r // NCH, r % NCH
        key = "ytq_a" if b == 0 else "ytq_b"
        y[b, CHUNK * c:CHUNK * (c + 1), :] = \
            res.results[r][key].astype(np.float32).T
    return y
